# revision 10
# baseline (speedup 1.0000x reference)
"""Trainium2 Bass kernel for nn_AttentionBlock (GroupNorm + 1x1 conv QKV + MHA + out-proj + residual).

Sharding: 8 cores = 2 batches x 4 heads. Each core computes GroupNorm for its
batch (duplicated), the qkv projection rows for its head, full [4096 x 4096]
attention for its (batch, head), and the partial output projection
w_out[:, head] @ a (unnormalized by the softmax denominator Z). The host
divides by Z, sums the 4 head partials per batch, and adds b_out + residual.

Device math notes:
  - softmax computed without max-subtraction (scores are bounded ~|7|, exp is
    safe in fp32); scale 1/sqrt(sqrt(ch)) folded into the q/k weights on host.
  - scores are computed directly in [s, t] orientation (S2 = k^T q) so the
    softmax contraction dim s lands on partitions for the AV matmul.
  - Z obtained for free via a ones-column appended to v^T (65th matmul column).
  - GroupNorm group stats via per-channel bn_stats + group-mask matmul.
"""

import os
import sys

import numpy as np

if os.path.isdir("/opt/trn_rl_repo") and "/opt/trn_rl_repo" not in sys.path:
    sys.path.insert(0, "/opt/trn_rl_repo")

import concourse.bass as bass
import concourse.mybir as mybir
import concourse.tile as tile
from concourse import bacc
from concourse.bass import ts

P = 128
L = 4096          # D*H*W
T = 512           # t-chunk size
NCHUNK = L // T   # 8
NST = L // P      # 32 s-tiles
CH = 64           # head dim
EPS = 1e-6
F32 = mybir.dt.float32
F32R = mybir.dt.float32r
N_CORES = 8


def build_attention_nc():
    """Build the single-core SPMD Bass program."""
    from contextlib import ExitStack

    nc = bacc.Bacc("TRN2", target_bir_lowering=False, debug=False, num_devices=N_CORES)
    AF = mybir.ActivationFunctionType
    OP = mybir.AluOpType

    xin = nc.dram_tensor("xin", [P, 2, L], F32, kind="ExternalInput").ap()
    wqkvT = nc.dram_tensor("wqkvT", [P, 2, 320], F32, kind="ExternalInput").ap()
    bqk_d = nc.dram_tensor("bqk", [P, 2], F32, kind="ExternalInput").ap()
    bv_d = nc.dram_tensor("bv", [CH], F32, kind="ExternalInput").ap()
    woutT = nc.dram_tensor("woutT", [CH, 2, P], F32, kind="ExternalInput").ap()
    gnsc_d = nc.dram_tensor("gnsc", [P, 2], F32, kind="ExternalInput").ap()
    gnbi_d = nc.dram_tensor("gnbi", [P, 2], F32, kind="ExternalInput").ap()
    gmask_d = nc.dram_tensor("gmask_in", [P, 8], F32, kind="ExternalInput").ap()
    yp_d = nc.dram_tensor("yp", [P, 2, L], F32, kind="ExternalOutput").ap()
    z_d = nc.dram_tensor("zout", [1, L], F32, kind="ExternalOutput").ap()

    with tile.TileContext(nc) as tc, ExitStack() as ctx:
        big = ctx.enter_context(tc.tile_pool(name="big", bufs=2))
        persist = ctx.enter_context(tc.tile_pool(name="persist", bufs=1))
        small = ctx.enter_context(tc.tile_pool(name="small", bufs=1))
        work = ctx.enter_context(tc.tile_pool(name="work", bufs=2))

        # ---- persistent tiles ----
        xt = big.tile([P, 2, L], F32, tag="big", name="xt")
        q2 = persist.tile([P, L], F32R, name="q2")          # q on parts 0:64, dup on 64:128
        k2 = persist.tile([P, L], F32R, name="k2")          # k on parts 64:128, dup on 0:64
        vt = persist.tile([P, NST, CH + 1], F32R, name="vt")  # v^T blocks + ones col
        wq_raw = persist.tile([P, 2, 320], F32, name="wq_raw")
        wq_sb = persist.tile([P, 2, 320], F32R, name="wq_sb")
        wo_raw = persist.tile([CH, 2, P], F32, name="wo_raw")
        wo_sb = persist.tile([CH, 2, P], F32R, name="wo_sb")
        gmask_raw = persist.tile([P, 8], F32, name="gmask_raw")
        bqk_sb = persist.tile([P, 2], F32, name="bqk_sb")
        bvbc = persist.tile([P, CH], F32, name="bvbc")
        gnsc_sb = persist.tile([P, 2], F32, name="gnsc_sb")
        gnbi_sb = persist.tile([P, 2], F32, name="gnbi_sb")
        gmask = persist.tile([P, 8], F32, name="gmask")

        # ---- input DMAs ----
        for po in range(2):
            for hh in range(2):
                nc.sync.dma_start(xt[:, po, ts(hh, L // 2)],
                                  xin[:, po, ts(hh, L // 2)])
        nc.sync.dma_start(wq_raw, wqkvT)
        nc.sync.dma_start(wo_raw, woutT)
        nc.vector.tensor_copy(wq_sb, wq_raw)
        nc.vector.tensor_copy(wo_sb, wo_raw)
        nc.sync.dma_start(bqk_sb, bqk_d)
        # broadcast bv [64] across all 128 partitions
        bv_bcast = bass.AP(tensor=bv_d.tensor, offset=bv_d.offset,
                           ap=[[0, P]] + list(bv_d.ap))
        nc.sync.dma_start(bvbc, bv_bcast)
        nc.sync.dma_start(gnsc_sb, gnsc_d)
        nc.sync.dma_start(gnbi_sb, gnbi_d)

        nc.sync.dma_start(gmask_raw, gmask_d)
        nc.vector.tensor_copy(gmask, gmask_raw)
        # ones column of vt (f32r memset is invalid ISA; use 0*x+1 instead)
        nc.vector.tensor_scalar(vt[:, :, CH:CH + 1],
                                bvbc[:, 0:NST].rearrange("p a -> p a ()"),
                                0.0, 1.0, OP.mult, OP.add)

        # ---- GroupNorm stats ----
        stats = small.tile([P, 2, 8, 6], F32, name="stats")
        mv = small.tile([P, 2, 2], F32, name="mv")
        for po in range(2):
            for i in range(8):
                nc.vector.bn_stats(stats[:, po, i, :], xt[:, po, ts(i, 512)])
            nc.vector.bn_aggr(mv[:, po, :], stats[:, po, :, :])
        rhs_gs = small.tile([P, 4], F32, name="rhs_gs")   # [m0 m1 s0 s1]
        nc.vector.tensor_copy(rhs_gs[:, 0:2], mv[:, :, 0])
        nc.vector.tensor_tensor(rhs_gs[:, 2:4], mv[:, :, 0], mv[:, :, 0], OP.mult)
        nc.vector.tensor_tensor(rhs_gs[:, 2:4], rhs_gs[:, 2:4], mv[:, :, 1], OP.add)

        with tc.tile_pool(name="pre_ps", bufs=2, space="PSUM") as pre_ps:
            # group sums: [8, 4] = gmask.T @ rhs_gs
            psg = pre_ps.tile([8, 4], F32, tag="g", name="psg")
            nc.tensor.matmul(psg, gmask, rhs_gs, start=True, stop=True)
            mg = small.tile([8, 2], F32, name="mg")
            varg = small.tile([8, 2], F32, name="varg")
            rstd = small.tile([8, 2], F32, name="rstd")
            tmp8 = small.tile([8, 2], F32, name="tmp8")
            epst = small.tile([8, 1], F32, name="epst")
            nc.vector.memset(epst, EPS)
            nc.vector.tensor_scalar_mul(mg, psg[:, 0:2], 1.0 / 16.0)
            nc.vector.tensor_scalar_mul(varg, psg[:, 2:4], 1.0 / 16.0)
            nc.vector.tensor_tensor(tmp8, mg, mg, OP.mult)
            nc.vector.tensor_tensor(varg, varg, tmp8, OP.subtract)
            nc.scalar.activation(varg, varg, AF.Sqrt, bias=epst)  # sqrt(var+eps)
            nc.vector.reciprocal(rstd, varg)
            # touch Exp now so the ACT table set loads during the pre-phase
            warmup = small.tile([8, 1], F32, name="warmup")
            nc.scalar.activation(warmup, epst, AF.Exp)

            # broadcast group stats to channels: [8,2] -> [128,2] (repeat 16x)
            rstdc = small.tile([P, 2], F32, name="rstdc")
            mgc = small.tile([P, 2], F32, name="mgc")
            for src, dst in ((rstd, rstdc), (mg, mgc)):
                rep = bass.AP(tensor=src.tensor, offset=src.offset,
                              ap=[list(src.ap[0]), [0, 16], list(src.ap[1])])
                nc.sync.dma_start(dst, rep)
            a_aff = small.tile([P, 2], F32, name="a_aff")
            b_aff = small.tile([P, 2], F32, name="b_aff")
            tmpc = small.tile([P, 2], F32, name="tmpc")
            nc.vector.tensor_tensor(a_aff, rstdc, gnsc_sb, OP.mult)
            nc.vector.tensor_tensor(tmpc, mgc, a_aff, OP.mult)
            nc.vector.tensor_tensor(b_aff, gnbi_sb, tmpc, OP.subtract)
            # xn = x*A + B (f32r output for the matmuls)
            xn = big.tile([P, 2, L], F32R, tag="big", name="xn")
            for po in range(2):
                nc.vector.tensor_scalar(xn[:, po, :], xt[:, po, :],
                                        a_aff[:, po:po + 1], b_aff[:, po:po + 1],
                                        OP.mult, OP.add)

            # ---- qkv projection ----
            # v^T blocks first: vt[:, j, 0:64] = xn[:, jP:(j+1)P]^T @ Wv^T + bv
            for j in range(NST):
                ps_vt = pre_ps.tile([P, CH], F32, tag="vt", name="ps_vt")
                for ko in range(2):
                    nc.tensor.matmul(ps_vt, xn[:, ko, ts(j, P)], wq_sb[:, ko, 128:192],
                                     start=(ko == 0), stop=(ko == 1))
                nc.vector.tensor_tensor(vt[:, j, 0:CH], ps_vt, bvbc, OP.add)
            # qk: normal layout [q;k] and swapped [k;q] so all four half-writes
            # are partition-aligned DVE copies (q/k duplicated on both halves)
            for ic in range(NCHUNK):
                ps_qk = pre_ps.tile([P, T], F32, tag="qk", name="ps_qk")
                ps_kq = pre_ps.tile([P, T], F32, tag="qk", name="ps_kq")
                for ko in range(2):
                    nc.tensor.matmul(ps_qk, wq_sb[:, ko, 0:128], xn[:, ko, ts(ic, T)],
                                     start=(ko == 0), stop=(ko == 1))
                for ko in range(2):
                    nc.tensor.matmul(ps_kq, wq_sb[:, ko, 192:320], xn[:, ko, ts(ic, T)],
                                     start=(ko == 0), stop=(ko == 1))
                nc.vector.tensor_scalar_add(q2[0:CH, ts(ic, T)], ps_qk[0:CH, :],
                                            bqk_sb[0:CH, 0:1])
                nc.vector.tensor_scalar_add(k2[CH:P, ts(ic, T)], ps_qk[CH:P, :],
                                            bqk_sb[CH:P, 0:1])
                nc.vector.tensor_scalar_add(k2[0:CH, ts(ic, T)], ps_kq[0:CH, :],
                                            bqk_sb[0:CH, 1:2])
                nc.vector.tensor_scalar_add(q2[CH:P, ts(ic, T)], ps_kq[CH:P, :],
                                            bqk_sb[CH:P, 1:2])

        # ---- attention ----
        # software pipeline: emit S2+exp for chunk ic, then AV/proj for chunk
        # ic-1, so the scheduler keeps ScalarE (exp, the bottleneck) fed while
        # PE drains the previous chunk's AV.
        with tc.tile_pool(name="att_s", bufs=2, space="PSUM") as sps, \
                tc.tile_pool(name="att_a", bufs=1, space="PSUM") as aps, \
                tc.tile_pool(name="att_y", bufs=1, space="PSUM") as yps:
            e2s = {}

            def emit_s2_exp(ic):
                e2 = big.tile([P, NST, T], F32R, tag="big", name="e2")
                e2s[ic] = e2
                gstart = 0
                while gstart < NST:
                    gsize = min(3, NST - gstart)
                    ps_s = sps.tile([P, 3, T], F32, tag="s", name="ps_s")
                    for jj in range(gsize):
                        sj = gstart + jj
                        hb = (sj % 2) * CH
                        nc.tensor.matmul(ps_s[:, jj, :],
                                         k2[hb:hb + CH, ts(sj, P)],
                                         q2[hb:hb + CH, ts(ic, T)],
                                         start=True, stop=True,
                                         tile_position=(hb, 0))
                    nc.scalar.activation(e2[:, gstart:gstart + gsize, :],
                                         ps_s[:, 0:gsize, :], AF.Exp)
                    gstart += gsize

            def emit_av_proj(ic):
                e2 = e2s.pop(ic)
                ps_a = aps.tile([P, T], F32, tag="a", name="ps_a")
                for j in range(NST):
                    nc.tensor.matmul(ps_a[0:CH + 1, :], vt[:, j, :], e2[:, j, :],
                                     start=(j == 0), stop=(j == NST - 1))
                azt = work.tile([CH + 1, T], F32R, tag="az", name="azt")
                nc.vector.tensor_copy(azt, ps_a[0:CH + 1, :])
                zt = work.tile([CH + 1, T], F32, tag="zt", name="zt")
                nc.vector.tensor_copy(zt[CH:CH + 1, :], ps_a[CH:CH + 1, :])
                nc.sync.dma_start(z_d[0:1, ts(ic, T)], zt[CH:CH + 1, :])
                ysb = work.tile([P, 2, T], F32, tag="y", name="ysb")
                for mo in range(2):
                    ps_y = yps.tile([P, T], F32, tag="y", name="ps_y")
                    nc.tensor.matmul(ps_y, wo_sb[:, mo, :], azt[0:CH, :],
                                     start=True, stop=True)
                    nc.vector.tensor_copy(ysb[:, mo, :], ps_y)
                nc.sync.dma_start(yp_d[:, :, ts(ic, T)], ysb)

            for ic in range(NCHUNK + 1):
                if ic < NCHUNK:
                    emit_s2_exp(ic)
                if ic >= 1:
                    emit_av_proj(ic - 1)

    nc.compile()
    return nc


def make_core_inputs(x, gn_scale, gn_bias, w_qkv, b_qkv, w_out, b_out):
    """Shard full inputs into 8 per-core input maps (batch n, head h)."""
    N, C, D, H, W = x.shape
    l = D * H * W
    xf = np.ascontiguousarray(x.reshape(N, C, l), dtype=np.float32)
    scale = np.float32(1.0 / np.sqrt(np.sqrt(CH)))
    gnsc = np.ascontiguousarray(gn_scale.reshape(2, P).T, dtype=np.float32)
    gnbi = np.ascontiguousarray(gn_bias.reshape(2, P).T, dtype=np.float32)
    in_maps = []
    for core in range(N_CORES):
        n, h = divmod(core, 4)
        xn_ = np.ascontiguousarray(
            xf[n].reshape(2, P, l).transpose(1, 0, 2))
        wq_h = w_qkv[h * CH:(h + 1) * CH] * scale
        wk_h = w_qkv[C + h * CH:C + (h + 1) * CH] * scale
        wv_h = w_qkv[2 * C + h * CH:2 * C + (h + 1) * CH]
        rows = np.concatenate([wq_h, wk_h, wv_h, wk_h, wq_h], axis=0)  # [320, 256]
        wq = np.ascontiguousarray(
            rows.T.reshape(2, P, 320).transpose(1, 0, 2), dtype=np.float32)
        bq_h = b_qkv[h * CH:(h + 1) * CH] * scale
        bk_h = b_qkv[C + h * CH:C + (h + 1) * CH] * scale
        bqk = np.stack([np.concatenate([bq_h, bk_h]),
                        np.concatenate([bk_h, bq_h])], axis=1).astype(np.float32)
        bv = np.ascontiguousarray(b_qkv[2 * C + h * CH:2 * C + (h + 1) * CH],
                                  dtype=np.float32)
        wo = np.ascontiguousarray(
            w_out[:, h * CH:(h + 1) * CH].T.reshape(CH, 2, P), dtype=np.float32)
        gm = np.zeros((P, 8), np.float32)
        for g in range(8):
            gm[g * 16:(g + 1) * 16, g] = 1.0
        in_maps.append({
            "xin": xn_, "wqkvT": wq, "bqk": np.ascontiguousarray(bqk),
            "bv": bv, "woutT": wo, "gnsc": gnsc, "gnbi": gnbi, "gmask_in": gm,
        })
    return in_maps


def combine_outputs(results, x, b_out):
    """Host gather: y = sum_h yp/z per batch + b_out + residual."""
    N, C, D, H, W = x.shape
    l = D * H * W
    xf = x.reshape(N, C, l)
    y = np.zeros((N, C, l), np.float32)
    for core, res in enumerate(results):
        n = core // 4
        yp = res["yp"].reshape(P, 2, l).transpose(1, 0, 2).reshape(C, l)
        z = res["zout"].reshape(l)
        y[n] += yp / z[None, :]
    y += b_out.astype(np.float32)[None, :, None] + xf
    return y.reshape(N, C, D, H, W).astype(np.float32)


_NC_CACHE = {}


def get_nc():
    if "nc" not in _NC_CACHE:
        _NC_CACHE["nc"] = build_attention_nc()
    return _NC_CACHE["nc"]


def kernel(x, gn_scale, gn_bias, w_qkv, b_qkv, w_out, b_out, _trace=False):
    from concourse.bass_utils import run_bass_kernel_spmd
    x = np.asarray(x); gn_scale = np.asarray(gn_scale); gn_bias = np.asarray(gn_bias)
    w_qkv = np.asarray(w_qkv); b_qkv = np.asarray(b_qkv)
    w_out = np.asarray(w_out); b_out = np.asarray(b_out)
    nc = get_nc()
    in_maps = make_core_inputs(x, gn_scale, gn_bias, w_qkv, b_qkv, w_out, b_out)
    res = run_bass_kernel_spmd(nc, in_maps, core_ids=list(range(N_CORES)),
                               trace=_trace)
    out = combine_outputs(res.results, x, b_out)
    if _trace:
        kernel.last_results = res
    return out


if __name__ == "__main__":
    sys.path.insert(0, os.path.dirname(os.path.abspath(__file__)))
    import reference
    inputs = {k: np.asarray(v) for k, v in reference.setup_inputs().items()}
    expected = np.asarray(reference.reference(**inputs))
    got = kernel(**inputs)
    err = np.abs(got - expected).max()
    rel = err / np.abs(expected).max()
    print("abs err:", err, "rel err:", rel)
